# revision 13
# baseline (speedup 1.0000x reference)
"""Multi-head Latent Attention (MLA) forward for Trainium2, 8 NeuronCores.

Sharding: core = (batch b in {0,1}) x (head-group of 4 heads out of 16).
Phase 1 (latent down-proj + LayerNorm) is token-sharded within each batch
group of 4 cores (512 tokens per core) and the normalized transposed
latents are exchanged with a single on-chip AllGather (1 MB/rank, bf16).
Phase 2 computes q/k/v up-projections for the core's 4 heads; Phase 3 runs
causal attention in a transposed sT[k, q] layout (kc-outer loop, scores
for the two active query groups share one PSUM tile so one Exp covers
both) and a partial output projection. The host sums 4 partials per batch.

All matmul operands are bf16 (fp32 PSUM accumulation). bf16 keeps the PE
at full rate while making LDWEIGHTS a separate, overlappable instruction
(fp32r matmuls must self-load weights serially) and halving DMA/SBUF.
Softmax denominators come from an all-ones [128,128] stationary operand,
which lands the denominator on every output partition so no reciprocal
row-broadcast is needed. x is transposed on the host so no PE transposes
of x are needed on device.
"""

import os

import numpy as np

B, S, D, H, HD, L = 2, 2048, 2048, 16, 128, 512
HPC = 4  # heads per core
NCORES = 8
SCALE = 1.0 / np.sqrt(128.0)
EPS = 1e-5
NEG = -1.0e9
NT = S // 128  # 16 token sub-tiles
LC = L // 128  # 4 latent chunks
DC = D // 128  # 16 feature chunks
TPC = S // 4  # 512 tokens per core in phase 1

_CACHE = {}
LAST = {}


def _build(has_down_bias, has_ln_affine, has_up_bias, has_mask, use_ag=True):
    import contextlib

    import concourse.bass as bass  # noqa: F401
    import concourse.tile as tile
    from concourse import bacc, mybir
    from concourse.masks import make_identity

    dt = mybir.dt
    f32 = dt.float32
    bf16 = dt.bfloat16
    ACT = mybir.ActivationFunctionType

    nc = bacc.Bacc("TRN2", target_bir_lowering=False, debug=False, num_devices=8)

    def din(name, shape, dtype=bf16):
        return nc.dram_tensor(name, shape, dtype, kind="ExternalInput").ap()

    xt_d = din("xt", [D, TPC])          # this core's token-slice of x^T
    wqd_d = din("wqd", [D, L])
    wkvd_d = din("wkvd", [D, L])
    wqu_d = din("wqu", [L, HPC * HD])   # pre-scaled by SCALE on host
    wku_d = din("wku", [L, HPC * HD])
    wvu_d = din("wvu", [L, HPC * HD])
    wo_d = din("wo", [HPC * HD, D])
    if has_mask:
        kbias_d = din("kbias", [128, NT], f32)
    if has_down_bias:
        bqd_d = din("bqd", [1, L], f32)
        bkvd_d = din("bkvd", [1, L], f32)
    if has_ln_affine:
        gq_d = din("gq", [1, L], f32)
        bq_d = din("bq", [1, L], f32)
        gkv_d = din("gkv", [1, L], f32)
        bkv_d = din("bkv", [1, L], f32)
    if has_up_bias:
        bqu_d = din("bqu", [128, HPC], f32)  # pre-scaled by SCALE on host
        bku_d = din("bku", [128, HPC], f32)
        bvu_d = din("bvu", [1, HPC * HD], f32)
    out_d = nc.dram_tensor("out", [S, D], bf16, kind="ExternalOutput").ap()

    with tile.TileContext(nc) as tc:
        with contextlib.ExitStack() as ctx:
            ctx.enter_context(
                nc.allow_low_precision(reason="bf16 matmuls are intentional")
            )
            const = ctx.enter_context(tc.tile_pool(name="const", bufs=1))

            ident = const.tile([128, 128], bf16, tag="ident", name="ident")
            ones_blk = const.tile([128, 128], bf16, tag="ones_blk", name="ones_blk")
            with tc.tile_pool(name="tmpconst", bufs=1) as tmpc:
                ident_f = tmpc.tile([128, 128], f32, tag="ident_f", name="ident_f")
                make_identity(nc, ident_f[:])
                nc.vector.tensor_copy(ident[:], ident_f[:])
                ones_f = tmpc.tile([128, 128], f32, tag="ones_f", name="ones_f")
                nc.gpsimd.memset(ones_f[:], 1.0)
                nc.vector.tensor_copy(ones_blk[:], ones_f[:])

            eps_col = const.tile([128, 1], f32, tag="eps_col", name="eps_col")
            nc.gpsimd.memset(eps_col[:], EPS)

            # register holding 0.0 for the gpsimd diagonal-mask fill
            zero_reg = nc.gpsimd.to_reg(0.0)

            if has_mask:
                kbias = const.tile([128, NT], f32, tag="kbias", name="kbias")
                nc.sync.dma_start(kbias[:], kbias_d[:])
            if has_down_bias:
                bd_reps = {}
                for nm, dap in (("bqd", bqd_d), ("bkvd", bkvd_d)):
                    t = const.tile([128, L], f32, tag=f"rep_{nm}", name=f"rep_{nm}")
                    nc.sync.dma_start(t[:], dap.broadcast_to((128, L)))
                    bd_reps[nm] = t
            if has_ln_affine:
                reps = {}
                for nm, dap in (
                    ("gq", gq_d),
                    ("bq", bq_d),
                    ("gkv", gkv_d),
                    ("bkv", bkv_d),
                ):
                    t = const.tile([128, L], f32, tag=f"rep_{nm}", name=f"rep_{nm}")
                    nc.sync.dma_start(t[:], dap.broadcast_to((128, L)))
                    reps[nm] = t
            if has_up_bias:
                bqu_sb = const.tile([128, HPC], f32, tag="bqu", name="bqu")
                nc.sync.dma_start(bqu_sb[:], bqu_d[:])
                bku_sb = const.tile([128, HPC], f32, tag="bku", name="bku")
                nc.sync.dma_start(bku_sb[:], bku_d[:])
                bvu_rep = const.tile([128, HPC * HD], f32, tag="bvu", name="bvu")
                nc.sync.dma_start(bvu_rep[:], bvu_d.broadcast_to((128, HPC * HD)))

            # partition id -> (token group g) via per-core input is not
            # available; instead the host passes the SAME program to all
            # cores and the token-group is identified by which slice of
            # the AllGather output is local. The program itself is
            # core-independent: each core's own tokens are ALWAYS written
            # to the ag_in bounce and its latT columns are filled from
            # the AG result for ALL four groups (including its own), so
            # no core-specific indexing is needed on device.

            # persistent transposed latents: [128, LC*S] (chunk c at cols c*S)
            latp = ctx.enter_context(tc.tile_pool(name="latT", bufs=1))
            q_lat = latp.tile([128, LC * S], bf16, tag="q_lat", name="q_lat")
            kv_lat = latp.tile([128, LC * S], bf16, tag="kv_lat", name="kv_lat")

            if use_ag:
                dram = ctx.enter_context(tc.tile_pool(name="dram", bufs=1, space="DRAM"))
                ag_in = {}
                ag_out = {}
                for path in ("kv", "q"):
                    ag_in[path] = dram.tile(
                        [LC * 128, TPC], bf16, tag=f"agi_{path}", name=f"agi_{path}"
                    )
                    ag_out[path] = dram.tile(
                        [8 * LC * 128, TPC],
                        bf16,
                        tag=f"ago_{path}",
                        name=f"ago_{path}",
                        addr_space="Shared",
                    )

            # ------------- Phase 1: xT -> z -> LN -> latT(own) -------------
            p1 = ctx.enter_context(contextlib.ExitStack())
            wpool = p1.enter_context(tc.tile_pool(name="wdown", bufs=1))
            xtp = p1.enter_context(tc.tile_pool(name="xt", bufs=1))
            wqd_all = wpool.tile([128, DC * L], bf16, tag="wqd", name="wqd_all")
            wkvd_all = wpool.tile([128, DC * L], bf16, tag="wkvd", name="wkvd_all")
            xt_all = xtp.tile([128, DC * TPC], bf16, tag="xt", name="xt_all")
            nc.sync.dma_start(
                wqd_all[:].rearrange("p (c w) -> p c w", c=DC),
                wqd_d.rearrange("(c p) w -> p c w", p=128),
            )
            nc.sync.dma_start(
                wkvd_all[:].rearrange("p (c w) -> p c w", c=DC),
                wkvd_d.rearrange("(c p) w -> p c w", p=128),
            )
            nc.sync.dma_start(
                xt_all[:].rearrange("p (c w) -> p c w", c=DC),
                xt_d.rearrange("(c p) w -> p c w", p=128),
            )

            zpool = p1.enter_context(tc.tile_pool(name="zpsum", bufs=5, space="PSUM"))
            tpsum = p1.enter_context(tc.tile_pool(name="tpsum", bufs=3, space="PSUM"))
            latsb = p1.enter_context(tc.tile_pool(name="latsb", bufs=4))
            stats = p1.enter_context(tc.tile_pool(name="stats", bufs=8))

            nsub = TPC // 128  # 4 own-token sub-chunks
            pid = nc.sync.partition_id()
            conds = (pid < 4, pid >= 4)
            readbacks = []
            for path in ("kv", "q"):
                wref = wkvd_all if path == "kv" else wqd_all
                dst = kv_lat if path == "kv" else q_lat
                for s in range(nsub):
                    zp = zpool.tile([128, L], f32, tag="z", name=f"z{s}{path}")
                    for c in range(DC):
                        nc.tensor.matmul(
                            zp[:],
                            xt_all[:, c * TPC + s * 128 : c * TPC + (s + 1) * 128],
                            wref[:, c * L : (c + 1) * L],
                            start=(c == 0),
                            stop=(c == DC - 1),
                        )
                    if has_down_bias:
                        zsb = latsb.tile([128, L], f32, tag="zsb", name=f"zsb{s}{path}")
                        nc.vector.tensor_add(
                            zsb[:], zp[:], bd_reps["bqd" if path == "q" else "bkvd"][:]
                        )
                        zsrc = zsb
                    else:
                        zsrc = zp
                    st6 = stats.tile([128, 6], f32, tag="st6", name=f"st{s}{path}")
                    nc.vector.bn_stats(st6[:], zsrc[:])
                    mv = stats.tile([128, 2], f32, tag="mv", name=f"mv{s}{path}")
                    nc.vector.bn_aggr(mv[:], st6[:])
                    mean = mv[:, 0:1]
                    var = mv[:, 1:2]
                    sq = stats.tile([128, 1], f32, tag="sq", name=f"sq{s}{path}")
                    nc.scalar.activation(sq[:], var, ACT.Sqrt, bias=eps_col[:], scale=1.0)
                    rr = stats.tile([128, 1], f32, tag="rr", name=f"rr{s}{path}")
                    nc.vector.reciprocal(rr[:], sq[:])
                    nmr = stats.tile([128, 1], f32, tag="nmr", name=f"nm{s}{path}")
                    nc.vector.tensor_mul(nmr[:], mean, rr[:])
                    nmr2 = stats.tile([128, 1], f32, tag="nmr2", name=f"nn{s}{path}")
                    nc.vector.tensor_scalar_mul(nmr2[:], nmr[:], -1.0)
                    if has_ln_affine:
                        latf = latsb.tile([128, L], f32, tag="latf", name=f"lf{s}{path}")
                        nc.scalar.activation(
                            latf[:], zsrc[:], ACT.Identity, bias=nmr2[:], scale=rr[:]
                        )
                        g_t = reps["gq" if path == "q" else "gkv"]
                        b_t = reps["bq" if path == "q" else "bkv"]
                        lat2 = latsb.tile([128, L], f32, tag="lat2", name=f"l2{s}{path}")
                        nc.vector.tensor_mul(lat2[:], latf[:], g_t[:])
                        lat = latsb.tile([128, L], bf16, tag="lat", name=f"la{s}{path}")
                        nc.vector.tensor_add(lat[:], lat2[:], b_t[:])
                    else:
                        lat = latsb.tile([128, L], bf16, tag="lat", name=f"la{s}{path}")
                        nc.scalar.activation(
                            lat[:], zsrc[:], ACT.Identity, bias=nmr2[:], scale=rr[:]
                        )
                    pt = tpsum.tile([128, L], bf16, tag="pt", name=f"pt{s}{path}")
                    for c in range(LC):
                        nc.tensor.transpose(
                            pt[:, c * 128 : (c + 1) * 128],
                            lat[:, c * 128 : (c + 1) * 128],
                            ident[:],
                        )
                    for c in range(LC):
                        dsub = dst[:, c * S + s * 128 : c * S + (s + 1) * 128]
                        psrc = pt[:, c * 128 : (c + 1) * 128]
                        if c % 2 == 0:
                            nc.scalar.copy(dsub, psrc)
                        else:
                            nc.vector.tensor_copy(dsub, psrc)

                # 8-rank AllGather of this path's latents (both batches'
                # blocks; readback picks this core's batch via predicated
                # DMAs). kv goes first so k/v up-projections can overlap
                # the q-path compute and AllGather.
                if use_ag:
                    agi = ag_in[path]
                    ago = ag_out[path]
                    nc.sync.dma_start(
                        agi.rearrange("(c p) w -> p c w", p=128),
                        dst[:].rearrange("p (c s) -> p c s", c=LC)[:, :, 0:TPC],
                    )
                    nc.gpsimd.collective_compute(
                        "AllGather",
                        mybir.AluOpType.bypass,
                        replica_groups=[[0, 1, 2, 3, 4, 5, 6, 7]],
                        ins=[agi.opt()],
                        outs=[ago.opt()],
                    )
                    for r in range(4):
                        for half in range(2):
                            readbacks.append((dst, ago, r, half))

            # preload the Exp activation table while the collective drains
            dummy = stats.tile([128, 1], f32, tag="dummy", name="dummy")
            nc.scalar.activation(dummy[:], eps_col[:], ACT.Exp, bias=0.0, scale=1.0)

            # readbacks AFTER both collective triggers: keeps the sync queue
            # from head-of-line-blocking the second collective's bounce DMA
            for dst, ago, r, half in readbacks:
                block = half * 4 + r
                nc.sync.dma_start(
                    dst[:].rearrange("p (c s) -> p c s", c=LC)[
                        :, :, r * TPC : (r + 1) * TPC
                    ],
                    ago[block * LC * 128 : (block + 1) * LC * 128, :].rearrange(
                        "(c p) w -> p c w", p=128
                    ),
                    cond=conds[half],
                )

            p1.close()

            # ------------- Phase 2: up-projections -------------------------
            kqv = ctx.enter_context(tc.tile_pool(name="kqv", bufs=1))
            p2 = ctx.enter_context(contextlib.ExitStack())
            upw = p2.enter_context(tc.tile_pool(name="upw", bufs=1))
            w = HPC * HD
            upalls = {}
            for nm, dap in (("wqu", wqu_d), ("wku", wku_d), ("wvu", wvu_d)):
                t = upw.tile([128, LC * w], bf16, tag=nm, name=f"{nm}_all")
                nc.sync.dma_start(
                    t[:].rearrange("p (c w) -> p c w", c=LC),
                    dap.rearrange("(c p) w -> p c w", p=128),
                )
                upalls[nm] = t
            wqu_all, wku_all, wvu_all = upalls["wqu"], upalls["wku"], upalls["wvu"]

            qT = [
                kqv.tile([128, S], bf16, tag=f"qT{h}", name=f"qT{h}") for h in range(HPC)
            ]
            kT = [
                kqv.tile([128, S], bf16, tag=f"kT{h}", name=f"kT{h}") for h in range(HPC)
            ]
            vt = [
                kqv.tile([128, w], bf16, tag=f"vt{s}", name=f"vt{s}") for s in range(NT)
            ]
            uppsum = p2.enter_context(tc.tile_pool(name="uppsum", bufs=4, space="PSUM"))

            ncopy = 0
            for which in ("k", "v", "q"):
                for G in range(4):
                    if which == "v":
                        for si in range(4):
                            s = G * 4 + si
                            pp = uppsum.tile(
                                [128, 512], f32, tag="up", name=f"upv{s}"
                            )
                            for c in range(LC):
                                nc.tensor.matmul(
                                    pp[:],
                                    kv_lat[:, c * S + s * 128 : c * S + (s + 1) * 128],
                                    wvu_all[:, c * w : (c + 1) * w],
                                    start=(c == 0),
                                    stop=(c == LC - 1),
                                )
                            if has_up_bias:
                                nc.vector.tensor_add(vt[s][:], pp[:], bvu_rep[:])
                            else:
                                if ncopy % 2 == 0:
                                    nc.scalar.copy(vt[s][:], pp[:])
                                else:
                                    nc.vector.tensor_copy(vt[s][:], pp[:])
                                ncopy += 1
                        continue
                    wsb = wqu_all if which == "q" else wku_all
                    dstT = qT if which == "q" else kT
                    src_lat = q_lat if which == "q" else kv_lat
                    for h in range(HPC):
                        pp = uppsum.tile(
                            [128, 512], f32, tag="up", name=f"up{G}{h}{which}"
                        )
                        for c in range(LC):
                            nc.tensor.matmul(
                                pp[:],
                                wsb[:, c * w + h * HD : c * w + (h + 1) * HD],
                                src_lat[:, c * S + G * 512 : c * S + (G + 1) * 512],
                                start=(c == 0),
                                stop=(c == LC - 1),
                            )
                        dsub = dstT[h][:, G * 512 : (G + 1) * 512]
                        if has_up_bias:
                            bcol = (bqu_sb if which == "q" else bku_sb)[:, h : h + 1]
                            nc.scalar.activation(
                                dsub, pp[:], ACT.Identity, bias=bcol, scale=1.0
                            )
                        else:
                            if ncopy % 2 == 0:
                                nc.scalar.copy(dsub, pp[:])
                            else:
                                nc.vector.tensor_copy(dsub, pp[:])
                            ncopy += 1
            p2.close()

            # ------------- Phase 3: attention ------------------------------
            wopool = ctx.enter_context(tc.tile_pool(name="wo", bufs=1))
            wo_all = wopool.tile([128, HPC * D], bf16, tag="wo", name="wo_all")
            nc.sync.dma_start(
                wo_all[:].rearrange("p (c w) -> p c w", c=HPC),
                wo_d.rearrange("(c p) w -> p c w", p=128),
            )

            otn_pool = ctx.enter_context(tc.tile_pool(name="otn", bufs=1))
            otn = [
                [
                    otn_pool.tile([128, 512], bf16, tag=f"otn{G}{h}", name=f"otn{G}{h}")
                    for h in range(HPC)
                ]
                for G in range(4)
            ]

            p3 = ctx.enter_context(contextlib.ExitStack())
            spsum = p3.enter_context(tc.tile_pool(name="spsum", bufs=2, space="PSUM"))
            opsum = p3.enter_context(tc.tile_pool(name="opsum", bufs=2, space="PSUM"))
            dpsum = p3.enter_context(tc.tile_pool(name="dpsum", bufs=2, space="PSUM"))
            espool = p3.enter_context(tc.tile_pool(name="es", bufs=3))
            small = p3.enter_context(tc.tile_pool(name="small", bufs=4))

            ndve = 0
            for h in range(HPC):
                for Gh in range(2):
                    G0, G1 = 2 * Gh, 2 * Gh + 1
                    otp = {
                        G: opsum.tile([128, 512], f32, tag="ot", name=f"ot{h}{G}")
                        for G in (G0, G1)
                    }
                    den = {
                        G: dpsum.tile([128, 512], f32, tag="dn", name=f"dn{h}{G}")
                        for G in (G0, G1)
                    }
                    nkc = 4 * (G1 + 1)
                    for kc in range(nkc):
                        active = [G for G in (G0, G1) if kc < 4 * (G + 1)]
                        sp = spsum.tile([128, 1024], f32, tag="sc", name=f"sc{h}{Gh}{kc}")
                        regions = {}
                        for G in active:
                            j = max(0, kc - 4 * G)
                            a = (G - G0) * 512 + j * 128
                            b2 = (G - G0 + 1) * 512
                            regions[G] = (a, b2, j)
                            nc.tensor.matmul(
                                sp[:, a:b2],
                                kT[h][:, kc * 128 : (kc + 1) * 128],
                                qT[h][:, G * 512 + j * 128 : (G + 1) * 512],
                                start=True,
                                stop=True,
                            )
                        lo = min(a for (a, _, _) in regions.values())
                        hi = max(b for (_, b, _) in regions.values())
                        es = espool.tile([128, 1024], bf16, tag="es", name=f"es{h}{Gh}{kc}")
                        if has_mask:
                            nc.scalar.activation(
                                es[:, lo:hi],
                                sp[:, lo:hi],
                                ACT.Exp,
                                bias=kbias[:, kc : kc + 1],
                                scale=1.0,
                            )
                        else:
                            nc.scalar.activation(
                                es[:, lo:hi], sp[:, lo:hi], ACT.Exp, bias=0.0, scale=1.0
                            )
                        for G in active:
                            a, b2, j = regions[G]
                            if kc >= 4 * G:  # diagonal block: zero masked weights
                                nc.gpsimd.affine_select(
                                    out=es[:, a : a + 128],
                                    in_=es[:, a : a + 128],
                                    compare_op=mybir.AluOpType.is_ge,
                                    fill=zero_reg,
                                    base=0,
                                    pattern=[[1, 128]],
                                    channel_multiplier=-1,
                                )
                        for G in active:
                            a, b2, j = regions[G]
                            nc.tensor.matmul(
                                den[G][:, j * 128 : 512],
                                ones_blk[:],
                                es[:, a:b2],
                                start=(kc == 0),
                                stop=(kc == 4 * G + 3),
                            )
                        for G in active:
                            a, b2, j = regions[G]
                            nc.tensor.matmul(
                                otp[G][:, j * 128 : 512],
                                vt[kc][:, h * HD : (h + 1) * HD],
                                es[:, a:b2],
                                start=(kc == 0),
                                stop=(kc == 4 * G + 3),
                            )
                        # normalize G0 as soon as its accumulation closes so
                        # its PSUM tiles free up while G1's chunks still run
                        for G in active:
                            if kc == 4 * G + 3:
                                rec = small.tile(
                                    [128, 512], f32, tag="rec", name=f"rc{h}{G}"
                                )
                                nc.vector.reciprocal_approx_fast(rec[:], den[G][:])
                                nc.vector.tensor_mul(otn[G][h][:], otp[G][:], rec[:])

            p3.close()

            # ------------- Phase 4: output projection ----------------------
            p4 = ctx.enter_context(contextlib.ExitStack())
            fpsum = p4.enter_context(tc.tile_pool(name="fpsum", bufs=8, space="PSUM"))
            outsb = p4.enter_context(tc.tile_pool(name="outsb", bufs=4))
            nob = 0
            for G in range(4):
                for ls in range(4):
                    ops = [
                        fpsum.tile([128, 512], f32, tag="op", name=f"op{G}{ls}{jc}")
                        for jc in range(4)
                    ]
                    for hh in range(HPC):
                        lhs = otn[G][hh][:, ls * 128 : (ls + 1) * 128]
                        for jc in range(4):
                            nc.tensor.matmul(
                                ops[jc][:],
                                lhs,
                                wo_all[:, hh * D + jc * 512 : hh * D + (jc + 1) * 512],
                                start=(hh == 0),
                                stop=(hh == HPC - 1),
                            )
                    tok0 = G * 512 + ls * 128
                    ob = outsb.tile([128, D], bf16, tag="ob", name=f"ob{G}{ls}")
                    for jc in range(4):
                        if nob % 2 == 0:
                            nc.scalar.copy(ob[:, jc * 512 : (jc + 1) * 512], ops[jc][:])
                        else:
                            nc.vector.tensor_copy(
                                ob[:, jc * 512 : (jc + 1) * 512], ops[jc][:]
                            )
                        nob += 1
                    nc.scalar.dma_start(out_d[tok0 : tok0 + 128, :], ob[:])
            p4.close()

    nc.compile()
    return nc


def kernel(**inputs):
    import ml_dtypes

    from concourse.bass_utils import run_bass_kernel_spmd

    bf16 = ml_dtypes.bfloat16

    x = np.asarray(inputs["x"], np.float32)
    mask = np.asarray(inputs["mask"])
    wq_down = np.asarray(inputs["wq_down"], np.float32)
    bq_down = np.asarray(inputs["bq_down"], np.float32)
    gq_ln = np.asarray(inputs["gq_ln"], np.float32)
    bq_ln = np.asarray(inputs["bq_ln"], np.float32)
    wq_up = np.asarray(inputs["wq_up"], np.float32)
    bq_up = np.asarray(inputs["bq_up"], np.float32)
    wkv_down = np.asarray(inputs["wkv_down"], np.float32)
    bkv_down = np.asarray(inputs["bkv_down"], np.float32)
    gkv_ln = np.asarray(inputs["gkv_ln"], np.float32)
    bkv_ln = np.asarray(inputs["bkv_ln"], np.float32)
    wkv_up = np.asarray(inputs["wkv_up"], np.float32)
    bkv_up = np.asarray(inputs["bkv_up"], np.float32)
    w_out = np.asarray(inputs["w_out"], np.float32)
    b_out = np.asarray(inputs["b_out"], np.float32)

    has_down_bias = bool(np.any(bq_down) or np.any(bkv_down))
    has_ln_affine = bool(
        np.any(gq_ln != 1.0) or np.any(bq_ln) or np.any(gkv_ln != 1.0) or np.any(bkv_ln)
    )
    has_up_bias = bool(np.any(bq_up) or np.any(bkv_up))
    has_mask = bool(np.any(mask))
    use_ag = os.environ.get("MLA_NO_AG", "") == ""
    key = (has_down_bias, has_ln_affine, has_up_bias, has_mask, use_ag)
    if key not in _CACHE:
        _CACHE[key] = _build(*key)
    nc = _CACHE[key]

    wk_up = wkv_up[:, :D]
    wv_up = wkv_up[:, D:]
    bk_up = bkv_up[:D]
    bv_up = bkv_up[D:]

    wqd_b = np.ascontiguousarray(wq_down).astype(bf16)
    wkvd_b = np.ascontiguousarray(wkv_down).astype(bf16)
    xt_b = [np.ascontiguousarray(x[b].T).astype(bf16) for b in range(B)]

    in_maps = []
    for core in range(NCORES):
        b = core // 4
        g = core % 4
        hs = slice(g * HPC * HD, (g + 1) * HPC * HD)
        m = {
            "xt": np.ascontiguousarray(xt_b[b][:, g * TPC : (g + 1) * TPC]),
            "wqd": wqd_b,
            "wkvd": wkvd_b,
            "wqu": np.ascontiguousarray(wq_up[:, hs] * SCALE).astype(bf16),
            "wku": np.ascontiguousarray(wk_up[:, hs]).astype(bf16),
            "wvu": np.ascontiguousarray(wv_up[:, hs]).astype(bf16),
            "wo": np.ascontiguousarray(w_out[hs, :]).astype(bf16),
        }
        if has_mask:
            kb = np.where(mask[b], np.float32(NEG), np.float32(0.0)).astype(np.float32)
            m["kbias"] = np.ascontiguousarray(kb.reshape(NT, 128).T)
        if has_down_bias:
            m["bqd"] = bq_down.reshape(1, L).copy()
            m["bkvd"] = bkv_down.reshape(1, L).copy()
        if has_ln_affine:
            m["gq"] = gq_ln.reshape(1, L).copy()
            m["bq"] = bq_ln.reshape(1, L).copy()
            m["gkv"] = gkv_ln.reshape(1, L).copy()
            m["bkv"] = bkv_ln.reshape(1, L).copy()
        if has_up_bias:
            m["bqu"] = np.ascontiguousarray(
                (bq_up[hs] * SCALE).reshape(HPC, 128).T.astype(np.float32)
            )
            m["bku"] = np.ascontiguousarray(bk_up[hs].reshape(HPC, 128).T)
            m["bvu"] = np.ascontiguousarray(bv_up[hs].reshape(1, HPC * HD))
        in_maps.append(m)

    trace = bool(os.environ.get("MLA_TRACE"))
    res = run_bass_kernel_spmd(nc, in_maps, core_ids=list(range(NCORES)), trace=trace)
    LAST["res"] = res
    partials = np.stack(
        [res.results[i]["out"].astype(np.float32) for i in range(NCORES)]
    )
    out = partials.reshape(B, 4, S, D).sum(axis=1) + b_out
    return out.astype(np.float32)
